# revision 23
# baseline (speedup 1.0000x reference)
"""Trainium2 Bass kernel for nn_GammaCapsGraph (capsule routing over gram matrix).

Math (per batch, X = x[b] of shape (D=128, N=1024)):
  G = X^T X (symmetric gram). All routing stats are computed FROM X ALONE,
  without materializing G:
    diag[n]  = G[n,n]        = colsum(X*X)
    rowsum[n]= sum_m G[n,m]  = s^T X,  s = free-dim rowsum of X
    ssq[n]   = sum_m G[n,m]^2 = x_n^T (X X^T) x_n = colsum(X * (K X)), K = X X^T
  K/Y run in bf16 on the PE (validated 5.4e-5 end-to-end rel err vs fp32's
  5.6e-6; threshold 2e-2). The colsums are matmuls with a ones/s lhsT into
  per-batch PSUM rows; a transpose pass packs them into column layout
  (128, 32) for the derived per-row stats (q, alpha, rr, d0).

  This makes the AllGather of (q|rr|d0) fire at ~t=15us instead of ~t=80us
  (the baseline needed G + a G*G elementwise pass first), after which all
  cores run routing iterations 1-2 redundantly on all 32 batches (only the
  global scalar t couples batches; its d0-term sum ships inside the AG
  payload). G itself is computed ONCE, in the output pass, and scaled
  directly from PSUM (v = a*G + c per row) - no SBUF staging of G at all.

Hardware pitfalls baked in: sqrt and exp live in different ACT LUT tables
(1.28us per swap) so sqrts are batched per table residency with warm-act
preloads in ACT-idle gaps; DVE has no divide/rsqrt; only Identity+accum_out
is safe on ACT; HWDGE queues exist only on sync+scalar.
"""
import os

import numpy as np

import concourse.bass as bass
import concourse.bacc as bacc
import concourse.tile as tile
import concourse.mybir as mybir
from concourse.bass_utils import run_bass_kernel_spmd

N_CORES = 8
B_LOC = 4
D = 128
N = 1024
NCH = 8  # column chunks of 128
NC32 = B_LOC * NCH  # 32 (b, ch) pairs
P_P = 0.9
NUM_SECONDARY = 1024
EPS = 1e-12
T_NUM = float(np.log(P_P * (NUM_SECONDARY - 1)) - np.log(1.0 - P_P))
C0 = 1.0 / N
TOTAL_ROWS = float(N_CORES * B_LOC * N)  # 32768 rows for the global d-mean

F = mybir.dt.float32
FR = mybir.dt.float32r
BF = mybir.dt.bfloat16
AF = mybir.ActivationFunctionType
OP = mybir.AluOpType
AX = mybir.AxisListType

LAST_EXEC_NS = None
_NC_CACHE = None
DEBUG_TAPS = False


def _build():
    nc = bacc.Bacc("TRN2", target_bir_lowering=False, debug=False,
                   enable_asserts=False, num_devices=N_CORES)
    xs = nc.dram_tensor("xs", (B_LOC, D, N), FR, kind="ExternalInput").ap()
    e30_in = nc.dram_tensor("e30", (D, 3), FR, kind="ExternalInput").ap()
    e31_in = nc.dram_tensor("e31", (D, 3), FR, kind="ExternalInput").ap()
    z12_in = nc.dram_tensor("z12", (D, 12), FR, kind="ExternalInput").ap()
    iden_in = nc.dram_tensor("iden", (D, D), F, kind="ExternalInput").ap()
    idenb_in = nc.dram_tensor("idenb", (D, D), BF, kind="ExternalInput").ap()
    m16_in = nc.dram_tensor("m16", (D, 16), F, kind="ExternalInput").ap()
    m16t_in = nc.dram_tensor("m16t", (16, D), F, kind="ExternalInput").ap()
    sel0_in = nc.dram_tensor("sel0", (D, NC32), F, kind="ExternalInput").ap()
    sel1_in = nc.dram_tensor("sel1", (D, NC32), F, kind="ExternalInput").ap()
    bcol_in = nc.dram_tensor("bcol4", (D, NC32), F, kind="ExternalInput").ap()
    bb2_in = nc.dram_tensor("bb2", (D, 256), F, kind="ExternalInput").ap()
    vout = nc.dram_tensor("v", (B_LOC, N, N), F, kind="ExternalOutput").ap()
    dbg = {}
    if DEBUG_TAPS:
        dbg["Sdbg"] = nc.dram_tensor("Sdbg", (3, B_LOC * N), F,
                                     kind="ExternalOutput").ap()
        for nm in ("qdbg", "alphadbg", "rrdbg", "d0dbg"):
            dbg[nm] = nc.dram_tensor(nm, (D, NC32), F,
                                     kind="ExternalOutput").ap()
        dbg["xTdbg"] = nc.dram_tensor("xTdbg", (D, NCH * D), BF,
                                      kind="ExternalOutput").ap()
        dbg["Kdbg"] = nc.dram_tensor("Kdbg", (D, D), BF,
                                     kind="ExternalOutput").ap()

    rg = [list(range(N_CORES))]

    with tile.TileContext(nc) as tc:
        with (
            tc.tile_pool(name="const", bufs=1) as cpool,
            tc.tile_pool(name="xp", bufs=1) as xp,
            tc.tile_pool(name="zz", bufs=2) as zz,
            tc.tile_pool(name="sp", bufs=1) as sp,
            tc.tile_pool(name="row", bufs=1) as row,
            tc.tile_pool(name="outp", bufs=4) as outp,
            tc.tile_pool(name="psb", bufs=2, space="PSUM") as psb,
            tc.tile_pool(name="pss", bufs=2, space="PSUM") as pss,
            tc.tile_pool(name="psP", bufs=1, space="PSUM") as psP,
            tc.tile_pool(name="dram", bufs=1, space="DRAM") as dram,
        ):
            _cnt = [0]

            def _nm(tag):
                _cnt[0] += 1
                return f"{tag}_{_cnt[0]}"

            # ---- tiles for constants; DMA issue order matters: x first ----
            ident = cpool.tile([D, D], F)
            identb = cpool.tile([D, D], BF)
            m16 = cpool.tile([D, 16], F)
            m16t = cpool.tile([16, D], F)
            sel0 = cpool.tile([D, NC32], F)
            sel1 = cpool.tile([D, NC32], F)
            bcol4 = cpool.tile([D, NC32], F)
            bb2 = cpool.tile([D, 256], F)
            ones128 = cpool.tile([D, 1], F)
            nc.vector.memset(ones128[:], 1.0)
            e30 = cpool.tile([D, 3], FR)
            e31 = cpool.tile([D, 3], FR)
            onesr = cpool.tile([1, D], F)
            nc.vector.memset(onesr[:], 1.0)

            # ACT-table warm helper: prepone table swaps into ACT-idle gaps
            warm = cpool.tile([1, 1], F)
            nc.vector.memset(warm[:], 1.0)
            warm_o = cpool.tile([1, 1], F)

            def warm_act(func, dep=None):
                # dep pins the table load after a producer; without it the
                # scheduler hoists the dep-free warm early and thrashes.
                src_ap = warm[:] if dep is None else dep
                nc.scalar.activation(warm_o[:], src_ap, func)

            # per-batch rowsum vector s lands in column 3b+2 of this
            # (128,12) masked lhsT (zero-initialized via DMA)
            z12 = sp.tile([D, 12], FR)

            # ---- x in: 2 HWDGE queues interleaved ----
            xf = [xp.tile([D, N], FR, tag=f"fx{b}", name=f"fx{b}")
                  for b in range(B_LOC)]
            nc.sync.dma_start(xf[0][:], xs[0])
            nc.scalar.dma_start(xf[1][:], xs[1])
            nc.sync.dma_start(xf[2][:], xs[2])
            nc.scalar.dma_start(xf[3][:], xs[3])
            # early constants (needed in phase A)
            nc.sync.dma_start(identb[:], idenb_in[:])
            nc.sync.dma_start(e30[:], e30_in[:])
            nc.sync.dma_start(e31[:], e31_in[:])
            nc.sync.dma_start(z12[:], z12_in[:])
            nc.sync.dma_start(ident[:], iden_in[:])
            nc.sync.dma_start(bcol4[:], bcol_in[:])

            warm_act(AF.Sqrt)  # phase-A derived stats use the sqrt table

            # ---- phase A: per-batch casts, transposes, K=XX^T, Y=KX ----

            x16 = [xp.tile([D, N], BF, tag=f"x16_{b}", name=f"x16_{b}")
                   for b in range(B_LOC)]
            xT16 = [xp.tile([D, NCH, D], BF, tag=f"xT{b}", name=f"xT{b}")
                    for b in range(B_LOC)]
            Zd = [zz.tile([D, N], FR, tag="zd", name=f"zd{b}")
                  for b in range(B_LOC)]
            for b in range(B_LOC):
                # bf16 cast + free-dim rowsum in one ACT pass
                with nc.allow_low_precision(reason="f32r accum == f32 bits"):
                    nc.scalar.activation(x16[b][:], xf[b][:], AF.Identity,
                                         accum_out=z12[:, 3 * b + 2:3 * b + 3])
                # PE transposes: xT16[p, ch, d] = X[d, 128ch+p]
                # (the xbar DMA transpose silently no-ops in this runtime)
                tp = psb.tile([D, N], BF, tag="big", name=_nm("tp"))
                for ch in range(NCH):
                    nc.tensor.transpose(tp[:, D * ch:D * (ch + 1)],
                                        x16[b][:, D * ch:D * (ch + 1)],
                                        identb[:])
                nc.vector.tensor_copy(xT16[b][:], tp[:])
                nc.gpsimd.tensor_tensor(Zd[b][:], xf[b][:], xf[b][:],
                                        op=OP.mult)

            # row-stat sums land in contiguous PSUM rows [0:3) by
            # accumulating three matmuls with column-masked lhsT:
            # [1,0,0]x=Zd -> row0=diag, [0,1,0]x=Z -> row1=ssq,
            # [0,0,s]x=X -> row2=rowsum. Engine partition windows must be
            # 32-aligned, so S keeps batches side-by-side in the free dim.
            S = sp.tile([3, B_LOC * N], F)
            # pack-transpose accumulator, filled per batch inside rowstats
            PTps = psP.tile([D, 96], F, tag="PT")

            def rowstats(b, Z):
                R = psb.tile([D, N], F, tag="big", name=_nm("R"))
                stats = [(e30, Zd[b]), (e31, Z),
                         (z12[:, 3 * b:3 * b + 3], xf[b])]
                for s_i, (lhs, rhs) in enumerate(stats):
                    for hh, csl in enumerate([slice(0, 512),
                                              slice(512, 1024)]):
                        nc.tensor.matmul(R[0:3, csl], lhs[:], rhs[:, csl],
                                         start=(s_i == 0), stop=(s_i == 2),
                                         skip_group_check=True)
                if b % 2 == 0:
                    nc.vector.tensor_copy(S[:, N * b:N * (b + 1)], R[0:3, :])
                else:
                    nc.scalar.copy(S[:, N * b:N * (b + 1)], R[0:3, :])
                for ch in range(NCH):
                    nc.tensor.transpose(
                        PTps[:, 12 * ch + 3 * b:12 * ch + 3 * b + 3],
                        S[:, N * b + D * ch:N * b + D * (ch + 1)],
                        ident[0:3, 0:3])

            Zs = []
            for b in range(B_LOC):
                Kps = pss.tile([D, D], F, tag="small", name=_nm("K"))
                for ch in range(NCH):
                    nc.tensor.matmul(Kps[:], xT16[b][:, ch, :],
                                     xT16[b][:, ch, :],
                                     start=(ch == 0), stop=(ch == NCH - 1))
                K16 = sp.tile([D, D], BF, tag=f"k16_{b}", name=f"k16_{b}")
                nc.scalar.copy(K16[:], Kps[:])
                Yps = psb.tile([D, N], F, tag="big")
                nc.tensor.matmul(Yps[:, 0:512], K16[:], x16[b][:, 0:512],
                                 start=True, stop=True)
                nc.tensor.matmul(Yps[:, 512:1024], K16[:],
                                 x16[b][:, 512:1024], start=True, stop=True)
                Z = zz.tile([D, N], FR, tag="z", name=_nm("z"))
                nc.vector.tensor_tensor(Z[:], xf[b][:], Yps[:], op=OP.mult)
                Zs.append(Z)
                if b >= 1:
                    rowstats(b - 1, Zs[b - 1])
            rowstats(B_LOC - 1, Zs[B_LOC - 1])

            if DEBUG_TAPS:
                nc.scalar.dma_start(dbg["Sdbg"][:], S[:])

            # ---- pack stats into column layout (128, 32), j = 8b+ch ----
            P = sp.tile([D, NCH, 12], F)
            nc.vector.tensor_copy(P[:], PTps[:])
            # b-major contiguous copies: view [p, ch, 3b+s] -> [p, b, ch]
            diag_c = sp.tile([D, NC32], F)
            ssq_c = sp.tile([D, NC32], F)
            rsum_c = sp.tile([D, NC32], F)
            for s_i, dst in [(0, diag_c), (1, ssq_c), (2, rsum_c)]:
                src = P[:, :, s_i:12:3].transpose([0, 2, 1])
                nc.vector.tensor_copy(dst[:], src)

            # ---- derived per-row stats (column layout) ----
            bbcol = sp.tile([D, NC32], F)
            nc.scalar.activation(bbcol[:], bcol4[:], AF.Square, scale=32.0)
            q = sp.tile([D, NC32], F)
            nc.vector.tensor_tensor(q[:], diag_c[:], ssq_c[:], op=OP.min)
            isq = sp.tile([D, NC32], F)
            nc.vector.reciprocal(isq[:], ssq_c[:])
            qr = sp.tile([D, NC32], F)
            nc.vector.tensor_tensor(qr[:], q[:], isq[:], op=OP.mult)
            alpha = sp.tile([D, NC32], F)
            nc.scalar.activation(alpha[:], qr[:], AF.Sqrt)
            rr = sp.tile([D, NC32], F)
            nc.vector.tensor_tensor(rr[:], alpha[:], rsum_c[:], op=OP.mult)
            nc.vector.tensor_tensor(rr[:], rr[:], bcol4[:], op=OP.mult)
            # iteration 0 (c = 1/N): d0
            sq0 = sp.tile([D, NC32], F)
            nc.vector.scalar_tensor_tensor(sq0[:], rr[:], 2.0 * C0, bbcol[:],
                                           op0=OP.mult, op1=OP.add)
            nc.vector.scalar_tensor_tensor(sq0[:], q[:], C0 * C0, sq0[:],
                                           op0=OP.mult, op1=OP.add)
            sqs0 = sp.tile([D, NC32], F)
            nc.scalar.activation(sqs0[:], sq0[:], AF.Sqrt)
            den0 = sp.tile([D, NC32], F)
            nc.vector.tensor_scalar_add(den0[:], sq0[:], 1.0)
            inv0 = sp.tile([D, NC32], F)
            nc.vector.reciprocal(inv0[:], den0[:])
            f0 = sp.tile([D, NC32], F)
            nc.vector.tensor_tensor(f0[:], sqs0[:], inv0[:], op=OP.mult)
            a1c = sp.tile([D, NC32], F)
            nc.vector.tensor_tensor(a1c[:], f0[:], sq0[:], op=OP.mult)
            nc.vector.tensor_tensor(a1c[:], a1c[:], f0[:], op=OP.mult)
            w0 = sp.tile([D, NC32], F)
            nc.vector.tensor_scalar(w0[:], f0[:], -2.0 * C0, 1.0,
                                    op0=OP.mult, op1=OP.add)
            a3c = sp.tile([D, NC32], F)
            nc.vector.tensor_tensor(a3c[:], w0[:], q[:], op=OP.mult)
            a4c = sp.tile([D, NC32], F)
            nc.vector.tensor_tensor(a4c[:], f0[:], rr[:], op=OP.mult)
            d2c = sp.tile([D, NC32], F)
            nc.vector.scalar_tensor_tensor(d2c[:], a4c[:], -2.0, a1c[:],
                                           op0=OP.mult, op1=OP.add)
            nc.vector.tensor_tensor(d2c[:], d2c[:], a3c[:], op=OP.add)
            d0c = sp.tile([D, NC32], F)
            nc.scalar.activation(d0c[:], d2c[:], AF.Sqrt)
            if DEBUG_TAPS:
                nc.scalar.dma_start(dbg["qdbg"][:], q[:])
                nc.scalar.dma_start(dbg["alphadbg"][:], alpha[:])
                nc.scalar.dma_start(dbg["rrdbg"][:], rr[:])
                nc.scalar.dma_start(dbg["d0dbg"][:], d0c[:])

            # local sum of d0 (ships inside the AG payload, row 0 col 384)
            pt0 = sp.tile([D, 1], F)
            nc.vector.reduce_sum(pt0[:], d0c[:], axis=AX.X)
            tot_ps = pss.tile([1, 1], F, tag="small", name=_nm("tot"))
            nc.tensor.matmul(tot_ps[:], ones128[:], pt0[:],
                             start=True, stop=True)
            tot_sb = sp.tile([1, 1], F)
            nc.scalar.copy(tot_sb[:], tot_ps[:])

            # ---- stage (16, 2, 386) and single AllGather ----
            stage = sp.tile([16, 2, 386], F)
            nc.vector.memset(stage[:, :, 384:386], 0.0)
            nc.vector.tensor_copy(stage[0:1, 0, 384:385], tot_sb[:])
            for h in range(2):
                for k, src in enumerate([q, rr, d0c]):
                    st_ps = pss.tile([16, D], F, tag="small", name=_nm("sT"))
                    nc.tensor.transpose(st_ps[:], src[:, 16 * h:16 * h + 16],
                                        ident[:])
                    eng = nc.vector if (h * 3 + k) % 2 == 0 else nc.scalar
                    if eng is nc.vector:
                        nc.vector.tensor_copy(
                            stage[:, h, D * k:D * (k + 1)], st_ps[:])
                    else:
                        nc.scalar.copy(stage[:, h, D * k:D * (k + 1)],
                                       st_ps[:])
            ag_in = dram.tile([16, 772], F, tag="agin")
            ag_out = dram.tile([D, 772], F, tag="agout", addr_space="Shared")
            nc.sync.dma_start(ag_in[:], stage[:])
            nc.gpsimd.collective_compute(
                "AllGather", OP.bypass, replica_groups=rg,
                ins=[ag_in.opt()], outs=[ag_out.opt()])
            # routing-time constants (sync queue is idle during the AG)
            nc.sync.dma_start(m16[:], m16_in[:])
            nc.sync.dma_start(m16t[:], m16t_in[:])
            nc.sync.dma_start(sel0[:], sel0_in[:])
            nc.sync.dma_start(sel1[:], sel1_in[:])
            nc.sync.dma_start(bb2[:], bb2_in[:])

            warm_act(AF.Exp, dep=stage[0:1, 0, 256:257])  # during AG wait

            # ---- gram prefill: 2 chunks run on PE during the AG wait ----
            gram_ps = {}

            def gram(i):
                b, ch = divmod(i, NCH)
                gps = psb.tile([D, N], F, tag="big")
                lhs = xf[b][:, D * ch:D * (ch + 1)]
                nc.tensor.matmul(gps[:, 0:512], lhs,
                                 xf[b][:, 0:512], start=True, stop=True)
                nc.tensor.matmul(gps[:, 512:1024], lhs,
                                 xf[b][:, 512:1024], start=True, stop=True)
                gram_ps[i] = gps

            gram(0)
            gram(1)

            # ---- TT load + routing (redundant, all 32 batches) ----
            TT = row.tile([D, 2, 386], F, tag="TT", name="TT")
            nc.scalar.dma_start(TT[:], ag_out[:])
            qm = TT[:, :, 0:D]
            rm = TT[:, :, D:256]
            d0m = TT[:, :, 256:384]

            def row_t(tag):
                return row.tile([D, 256], F, tag=tag, name=_nm(tag))

            def t_chain(sum_ap):
                """global t from a (p,1)/(128,.) partial-sum AP -> (128,1)"""
                tps = pss.tile([1, 1], F, tag="small", name=_nm("tps"))
                nc.tensor.matmul(tps[:], ones128[:], sum_ap,
                                 start=True, stop=True)
                tot = row.tile([1, 1], F, tag="tot", name=_nm("tot"))
                nc.scalar.copy(tot[:], tps[:])
                dent = row.tile([1, 1], F, tag="dent", name=_nm("dent"))
                nc.vector.tensor_scalar(dent[:], tot[:],
                                        -0.5 / TOTAL_ROWS, EPS,
                                        op0=OP.mult, op1=OP.add)
                it = row.tile([1, 1], F, tag="it", name=_nm("it"))
                nc.vector.reciprocal(it[:], dent[:])
                tv = row.tile([1, 1], F, tag="tv", name=_nm("tv"))
                nc.vector.tensor_scalar_mul(tv[:], it[:], T_NUM)
                tb_ps = pss.tile([D, 1], F, tag="small", name=_nm("tbps"))
                nc.tensor.matmul(tb_ps[:], onesr[:], tv[:],
                                 start=True, stop=True)
                tb = row.tile([D, 1], F, tag="tb", name=_nm("tb"))
                nc.scalar.copy(tb[:], tb_ps[:])
                return tb

            def softmax_c(d_h0, d_h1, tb):
                """c = softmax over each batch's 1024 rows of tb*d."""
                e = row_t("e")
                es = row.tile([D, 2], F, tag="es", name=_nm("es"))
                nc.scalar.activation(e[:, 0:D], d_h0, AF.Exp, scale=tb[:],
                                     accum_out=es[:, 0:1])
                nc.scalar.activation(e[:, D:256], d_h1, AF.Exp, scale=tb[:],
                                     accum_out=es[:, 1:2])
                bs_ps = pss.tile([16, 2], F, tag="small", name=_nm("bs"))
                nc.tensor.matmul(bs_ps[:], m16[:], es[:],
                                 start=True, stop=True)
                bs = row.tile([16, 2], F, tag="bs", name=_nm("bs"))
                nc.vector.tensor_copy(bs[:], bs_ps[:])
                binv = row.tile([16, 2], F, tag="binv", name=_nm("binv"))
                nc.vector.reciprocal(binv[:], bs[:])
                ib_ps = pss.tile([D, 2], F, tag="small", name=_nm("ib"))
                nc.tensor.matmul(ib_ps[:], m16t[:], binv[:],
                                 start=True, stop=True)
                ib = row.tile([D, 2], F, tag="ib", name=_nm("ib"))
                nc.vector.tensor_copy(ib[:], ib_ps[:])
                c = row_t("c")
                nc.vector.tensor_scalar(c[:, 0:D], e[:, 0:D],
                                        ib[:, 0:1], None, op0=OP.mult)
                nc.vector.tensor_scalar(c[:, D:256], e[:, D:256],
                                        ib[:, 1:2], None, op0=OP.mult)
                return c

            def compute_sq_f(c):
                """sq = c*(c*q + 2rr) + bb; f = sqrt(sq)/(1+sq)"""
                u = row_t("u")
                nc.vector.tensor_tensor(u[:], c[:], qm, op=OP.mult)
                nc.vector.scalar_tensor_tensor(u[:], rm, 2.0, u[:],
                                               op0=OP.mult, op1=OP.add)
                sq = row_t("sq")
                nc.vector.tensor_tensor(sq[:], c[:], u[:], op=OP.mult)
                nc.vector.tensor_tensor(sq[:], sq[:], bb2[:], op=OP.add)
                sqs = row_t("sqs")
                nc.scalar.activation(sqs[:], sq[:], AF.Sqrt)
                den = row_t("den")
                nc.vector.tensor_scalar_add(den[:], sq[:], 1.0)
                inv = row_t("inv")
                nc.vector.reciprocal(inv[:], den[:])
                f = row_t("f")
                nc.vector.tensor_tensor(f[:], sqs[:], inv[:], op=OP.mult)
                return sq, f

            # iteration 1
            tb0 = t_chain(TT[:, 0, 384:385])
            c1 = softmax_c(d0m[:, 0, :], d0m[:, 1, :], tb0)
            warm_act(AF.Sqrt, dep=c1[0:1, 0:1])  # while DVE runs sq1 chain
            sq1, f1 = compute_sq_f(c1)
            fc1 = row_t("fc1")
            nc.vector.tensor_tensor(fc1[:], f1[:], c1[:], op=OP.mult)
            w1 = row_t("w1")
            nc.vector.tensor_scalar(w1[:], fc1[:], -2.0, 1.0,
                                    op0=OP.mult, op1=OP.add)
            a1 = row_t("a1")
            nc.vector.tensor_tensor(a1[:], f1[:], sq1[:], op=OP.mult)
            nc.vector.tensor_tensor(a1[:], a1[:], f1[:], op=OP.mult)
            a3 = row_t("a3")
            nc.vector.tensor_tensor(a3[:], w1[:], qm, op=OP.mult)
            a4 = row_t("a4")
            nc.vector.tensor_tensor(a4[:], f1[:], rm, op=OP.mult)
            d2 = row_t("d2")
            nc.vector.scalar_tensor_tensor(d2[:], a4[:], -2.0, a1[:],
                                           op0=OP.mult, op1=OP.add)
            nc.vector.tensor_tensor(d2[:], d2[:], a3[:], op=OP.add)
            d1 = row_t("d1")
            ds1 = row.tile([D, 1], F, tag="ds1", name="ds1")
            nc.scalar.activation(d1[:], d2[:], AF.Sqrt, accum_out=ds1[:])
            warm_act(AF.Exp, dep=d1[0:1, 0:1])  # while t1 chain runs
            # iteration 2 (final): only c2, f2 needed
            tb1 = t_chain(ds1[:])
            c2 = softmax_c(d1[:, 0:D], d1[:, D:256], tb1)
            warm_act(AF.Sqrt, dep=c2[0:1, 0:1])
            _, f2 = compute_sq_f(c2)
            fc2 = row_t("fc2")
            nc.vector.tensor_tensor(fc2[:], f2[:], c2[:], op=OP.mult)

            # ---- extract our 4 batches to column layout via selection ----
            fcT_ps = pss.tile([D, NC32], F, tag="small", name=_nm("fcT"))
            nc.tensor.matmul(fcT_ps[:], fc2[:, 0:D], sel0[:],
                             start=True, stop=False)
            nc.tensor.matmul(fcT_ps[:], fc2[:, D:256], sel1[:],
                             start=False, stop=True)
            fcT = sp.tile([D, NC32], F)
            nc.scalar.copy(fcT[:], fcT_ps[:])
            fT_ps = pss.tile([D, NC32], F, tag="small", name=_nm("fT"))
            nc.tensor.matmul(fT_ps[:], f2[:, 0:D], sel0[:],
                             start=True, stop=False)
            nc.tensor.matmul(fT_ps[:], f2[:, D:256], sel1[:],
                             start=False, stop=True)
            fT = sp.tile([D, NC32], F)
            nc.vector.tensor_copy(fT[:], fT_ps[:])
            a_col = sp.tile([D, NC32], F)
            nc.vector.tensor_tensor(a_col[:], fcT[:], alpha[:], op=OP.mult)
            c_col = sp.tile([D, NC32], F)
            nc.vector.tensor_tensor(c_col[:], fT[:], bcol4[:], op=OP.mult)

            # ---- output: gram -> fused scale from PSUM -> stream out ----
            for i in range(NC32):
                b, ch = divmod(i, NCH)
                if i not in gram_ps:
                    gram(i)
                gps = gram_ps.pop(i)
                ot = outp.tile([D, N], F, tag="out")
                if i % 2 == 0:
                    nc.vector.tensor_scalar(ot[:], gps[:],
                                            a_col[:, i:i + 1],
                                            c_col[:, i:i + 1],
                                            op0=OP.mult, op1=OP.add)
                    nc.sync.dma_start(vout[b, D * ch:D * (ch + 1), :], ot[:])
                else:
                    nc.scalar.activation(ot[:], gps[:], AF.Identity,
                                         bias=c_col[:, i:i + 1],
                                         scale=a_col[:, i:i + 1])
                    nc.scalar.dma_start(vout[b, D * ch:D * (ch + 1), :],
                                        ot[:])

    nc.compile()
    return nc


def _get_nc():
    global _NC_CACHE
    if _NC_CACHE is None:
        _NC_CACHE = _build()
    return _NC_CACHE


def _make_host_inputs():
    iden = np.eye(D, dtype=np.float32)
    m16 = np.zeros((D, 16), dtype=np.float32)
    m16t = np.zeros((16, D), dtype=np.float32)
    for g in range(16):
        m16[8 * g:8 * g + 8, g] = 1.0
        m16t[g, 8 * g:8 * g + 8] = 1.0
    return iden, m16, m16t


def _make_sel(core):
    # TT rows: rank r block at 16r, row j = 8b'+ch (half 0: batches 0,1 /
    # half 1: batches 2,3). Column j_out = 8b+ch global.
    sel0 = np.zeros((D, NC32), dtype=np.float32)
    sel1 = np.zeros((D, NC32), dtype=np.float32)
    for b in range(B_LOC):
        for c in range(NCH):
            if b < 2:
                sel0[16 * core + 8 * b + c, 8 * b + c] = 1.0
            else:
                sel1[16 * core + 8 * (b - 2) + c, 8 * b + c] = 1.0
    return sel0, sel1


def _reference_numpy(x, bias):
    """General fallback (non-row-constant bias): straight numpy port."""
    x = x.astype(np.float32)
    bias = bias.astype(np.float32)
    u_norm = np.linalg.norm(x, axis=1)[..., None]
    u_hat = np.einsum('bdn,bdm->bnm', x, x)
    u_hat_norm = np.linalg.norm(u_hat, axis=-1, keepdims=True)
    new_norm = np.minimum(u_hat_norm, u_norm)
    u_hat = u_hat / u_hat_norm * new_norm
    t_num = np.float32(T_NUM)
    b_ij = np.zeros(u_hat.shape, dtype=np.float32)
    v_j = None
    for it in range(3):
        m = b_ij.max(axis=1, keepdims=True)
        e = np.exp(b_ij - m)
        c_ij = e / e.sum(axis=1, keepdims=True)
        s_j = c_ij * u_hat + bias
        sqn = np.sum(s_j * s_j, axis=-1, keepdims=True)
        v_j = sqn * s_j / ((1.0 + sqn) * np.sqrt(sqn))
        if it < 2:
            dd = np.linalg.norm(v_j - u_hat, axis=-1, keepdims=True)
            d_o = dd.mean()
            t = t_num / (0.5 * d_o - d_o + EPS)
            b_ij = t * dd
    return v_j


def kernel(x, bias):
    global LAST_EXEC_NS
    x = np.ascontiguousarray(x, dtype=np.float32)
    bias = np.ascontiguousarray(bias, dtype=np.float32)
    B = x.shape[0]
    row_const = bool((bias == bias[:, :, :1]).all())
    if not row_const or B != 32 or x.shape[1:] != (D, N):
        return _reference_numpy(x, bias)
    brow = np.ascontiguousarray(bias[0, :, 0]).astype(np.float32)  # (N,)
    iden, m16, m16t = _make_host_inputs()
    import ml_dtypes
    idenb = iden.astype(ml_dtypes.bfloat16)
    e30 = np.zeros((D, 3), dtype=np.float32); e30[:, 0] = 1.0
    e31 = np.zeros((D, 3), dtype=np.float32); e31[:, 1] = 1.0
    # bcol4[p, 8b+c] = bias[128c+p] (8-col block repeated per batch)
    bcol = brow.reshape(NCH, D).T  # (128, 8): [p, c]
    bcol4 = np.ascontiguousarray(np.tile(bcol, (1, B_LOC)))
    # bb2 row layout: rows 8g+c repeat the per-chunk pattern; both halves same
    bb_row = (32.0 * brow) ** 2  # N*bias^2, (N,)
    bbp = bb_row.reshape(NCH, D)  # [c, p]
    bb128 = np.zeros((D, D), dtype=np.float32)
    for g in range(16):
        bb128[8 * g:8 * g + 8, :] = bbp
    bb2 = np.ascontiguousarray(np.concatenate([bb128, bb128], axis=1))
    nc = _get_nc()
    in_maps = []
    for core in range(N_CORES):
        sel0, sel1 = _make_sel(core)
        in_maps.append({
            "xs": np.ascontiguousarray(x[4 * core:4 * core + 4]),
            "e30": e30, "e31": e31,
            "idenb": idenb,
            "z12": np.zeros((D, 12), dtype=np.float32),
            "iden": iden, "m16": m16, "m16t": m16t,
            "sel0": sel0, "sel1": sel1,
            "bcol4": bcol4, "bb2": bb2,
        })
    res = run_bass_kernel_spmd(nc, in_maps, core_ids=list(range(N_CORES)))
    LAST_EXEC_NS = res.exec_time_ns
    globals()["LAST_RES"] = res
    return np.concatenate([res.results[c]["v"] for c in range(N_CORES)], axis=0)
